# revision 1
# baseline (speedup 1.0000x reference)
"""DiffVG-style circle renderer on 8 Trainium2 NeuronCores.

Strategy: shard the 1024x1024 image by rows (128 rows per core). Each core
composites the circles whose vertical span intersects its row band,
front-to-back with transmittance T:

    cov = sigmoid(r - d^2/r)          ~= sigmoid(2(r - d)) near the edge
    w   = T * cov                      (w ring, fp16)
    T  += (-a) * w                     (transmittance chain)
    C_ch += (a*col_ch) * w             (via premultiplied m_ch ring)

Front-to-back order is relaxed: circles whose column windows don't overlap
commute, so each core emits a width-descending order compatible with the
z partial order. Slot k's window width is the max over cores of the k-th
emitted circle width (compile-time constant); offsets are runtime data.

Engine split (per-instruction overheads dominate, so ops are batched and
spread so each engine runs ~1.1-1.3us per slot):
  PE     z = (r^2 - d^2)/r outer-sum; two circles per K=8 matmul
         (bf16 hi/lo split operands), bias folded in -> no sqrt pass
  ACT    per-pair sigmoid PSUM -> fp16 cov ring; m_B premultiply (Copy)
  Pool   w = T*cov (dynamic window); m_G premultiply (tensor_scalar)
  DVE    T += (-a)*w (stt, chain-critical); m_R premultiply; single
         3-plane batched C add at a dynamic offset (Q3)
State T and [CR|CG|CB] are fp16 planes; output = 4 fp16 planes DMA'd out,
assembled/converted to f32 on host.
"""

import sys

if "/opt/trn_rl_repo" not in sys.path:
    sys.path.insert(0, "/opt/trn_rl_repo")

import numpy as np
import ml_dtypes

import concourse.bass as bass
import concourse.bacc as bacc
import concourse.mybir as mybir
from concourse.tile import TileContext
from concourse import bass_utils

H = 1024
W = 1024
ROWS = 128
N_CORES = 8
MARGIN = 5.0
ROUND = 8
WMIN = 24
WCAP = 224
F32 = mybir.dt.float32
F16 = mybir.dt.float16
BF16 = mybir.dt.bfloat16
I32 = mybir.dt.int32
AF = mybir.ActivationFunctionType
OP = mybir.AluOpType
BF = ml_dtypes.bfloat16


# ---------------------------------------------------------------- host plan
def _core_circles(centers, radii, core):
    """Kept circle indices + cap-clipped rounded widths + offsets."""
    y0 = ROWS * core
    cy = centers[:, 1].astype(np.float64)
    cx = centers[:, 0].astype(np.float64)
    r = radii.astype(np.float64)
    keep = (cy + r + MARGIN >= y0 + 0.5) & (cy - r - MARGIN <= y0 + ROWS - 0.5)
    idx = np.where(keep)[0]
    dymin = np.maximum(0.0, np.maximum(y0 + 0.5 - cy[idx],
                                       cy[idx] - (y0 + ROWS - 0.5)))
    rm = r[idx] + MARGIN
    halfw = np.sqrt(np.maximum(rm * rm - dymin * dymin, 4.0))
    ws = np.clip(np.ceil(2.0 * halfw / ROUND) * ROUND, WMIN, WCAP).astype(int)
    off = np.clip(np.round(cx[idx] - ws / 2.0), 0, W - ws).astype(int)
    return idx, ws, off


def _greedy_f2b(idx, ws, off):
    """Front-to-back (topmost first) order, widest-available-first among
    circles whose later-drawn column-overlapping circles are all emitted."""
    n = len(idx)
    lo, hi = off, off + ws
    done = np.zeros(n, bool)
    order = []
    for _ in range(n):
        best, bestw = -1, -1
        for j in range(n):
            if done[j]:
                continue
            ok = True
            for p in range(n):
                if p == j or done[p]:
                    continue
                if idx[p] > idx[j] and lo[p] < hi[j] and lo[j] < hi[p]:
                    ok = False
                    break
            if ok and ws[j] > bestw:
                bestw, best = ws[j], j
        order.append(best)
        done[best] = True
    return np.array(order, int)


def make_plan(centers, radii):
    """Per-core ordered circle lists + global slot width profile."""
    percore = []
    for core in range(N_CORES):
        idx, ws, off = _core_circles(centers, radii, core)
        o = _greedy_f2b(idx, ws, off)
        percore.append((idx[o], ws[o], off[o]))
    S = max(len(p[0]) for p in percore)
    S = ((S + 1) // 2) * 2

    # swap-pass: adjacent column-disjoint circles commute; swap where it
    # lowers the per-slot cross-core max-width profile
    P = np.zeros((N_CORES, S), int)
    for c, (ids, ws, off) in enumerate(percore):
        P[c, :len(ws)] = ws
    for _ in range(4):
        changed = False
        for c in range(N_CORES):
            ids, ws, off = percore[c]
            for k in range(len(ws) - 1):
                if not (off[k] + ws[k] <= off[k + 1]
                        or off[k + 1] + ws[k + 1] <= off[k]):
                    continue
                others = np.delete(P, c, axis=0)
                ok = max(int(others[:, k].max()), WMIN)
                ok1 = max(int(others[:, k + 1].max()), WMIN)
                cur = max(ok, ws[k]) + max(ok1, ws[k + 1])
                new = max(ok, ws[k + 1]) + max(ok1, ws[k])
                if new < cur:
                    ids[k], ids[k + 1] = ids[k + 1], ids[k]
                    ws[k], ws[k + 1] = ws[k + 1], ws[k]
                    off[k], off[k + 1] = off[k + 1], off[k]
                    P[c, k], P[c, k + 1] = ws[k], ws[k + 1]
                    changed = True
        if not changed:
            break

    slotw = np.full(S, WMIN, int)
    for idx, ws, off in percore:
        slotw[:len(ws)] = np.maximum(slotw[:len(ws)], ws)
    return percore, slotw


def _hilo(x):
    hi = x.astype(BF)
    lo = (x - hi.astype(np.float64)).astype(BF)
    return hi, lo


def make_inputs(centers, radii, colors, plan):
    percore, slotw = plan
    S = len(slotw)
    npairs = S // 2
    pairw = [int(slotw[2 * i] + slotw[2 * i + 1]) for i in range(npairs)]
    assert all(pw <= 448 for pw in pairw)
    rhs_len = sum(pairw)
    pair_start = np.concatenate([[0], np.cumsum(pairw)]).astype(int)

    cy = centers[:, 1].astype(np.float64)
    cx = centers[:, 0].astype(np.float64)
    r = radii.astype(np.float64)
    col = colors.astype(np.float64)

    ins = []
    for core in range(N_CORES):
        y0 = ROWS * core
        ids, ws, offs_c = percore[core]
        n = len(ids)
        scal = np.zeros((ROWS, S * 4), np.float32)
        offs = np.zeros((1, S), np.int32)
        lhsT = np.zeros((8, npairs * ROWS), BF)
        rhs = np.zeros((8, rhs_len), BF)
        p = y0 + np.arange(ROWS, dtype=np.float64) + 0.5
        for k in range(n):
            i = ids[k]
            vk = int(slotw[k])
            off = int(np.clip(offs_c[k] + (ws[k] - vk) // 2, 0, W - vk))
            offs[0, k] = off
            al = col[i, 3]
            scal[:, k * 4 + 0] = -al
            scal[:, k * 4 + 1] = al * col[i, 0]
            scal[:, k * 4 + 2] = al * col[i, 1]
            scal[:, k * 4 + 3] = al * col[i, 2]
            j = off + np.arange(vk, dtype=np.float64) + 0.5
            a = r[i] / 2.0 - (p - cy[i]) ** 2 / r[i]
            b = r[i] / 2.0 - (j - cx[i]) ** 2 / r[i]
            ah, alo = _hilo(a)
            bh, blo = _hilo(b)
            pair, half = divmod(k, 2)
            rb = 4 * half
            ls = slice(pair * ROWS, (pair + 1) * ROWS)
            lhsT[rb + 0, ls] = ah
            lhsT[rb + 1, ls] = alo
            lhsT[rb + 2, ls] = 1.0
            lhsT[rb + 3, ls] = 1.0
            c0 = pair_start[pair] + (0 if half == 0 else int(slotw[2 * pair]))
            rs = slice(c0, c0 + vk)
            rhs[rb + 0, rs] = 1.0
            rhs[rb + 1, rs] = 1.0
            rhs[rb + 2, rs] = bh
            rhs[rb + 3, rs] = blo
        ins.append({"scal": scal, "offs": offs, "lhsT": lhsT, "rhs": rhs})
    return ins


# ------------------------------------------------------------- device build
def build_nc(slotw):
    slotw = [int(v) for v in slotw]
    S = len(slotw)
    npairs = S // 2
    pairw = [slotw[2 * i] + slotw[2 * i + 1] for i in range(npairs)]
    pair_start = [0]
    for pw in pairw:
        pair_start.append(pair_start[-1] + pw)
    rhs_len = pair_start[-1]
    ngroups = (S + 7) // 8

    nc = bacc.Bacc("TRN2", target_bir_lowering=False, debug=False,
                   num_devices=N_CORES)
    scal_d = nc.dram_tensor("scal", [ROWS, S * 4], F32,
                            kind="ExternalInput").ap()
    offs_d = nc.dram_tensor("offs", [1, S], I32, kind="ExternalInput").ap()
    lhsT_d = nc.dram_tensor("lhsT", [8, npairs * ROWS], BF16,
                            kind="ExternalInput").ap()
    rhs_d = nc.dram_tensor("rhs", [8, rhs_len], BF16,
                           kind="ExternalInput").ap()
    out_d = nc.dram_tensor("out", [ROWS, 4 * W], F16,
                           kind="ExternalOutput").ap()

    with TileContext(nc) as tc:
        T = nc.alloc_sbuf_tensor("T", [ROWS, W], F16).ap()
        CC = nc.alloc_sbuf_tensor("CC", [ROWS, 3 * W], F16).ap()
        AT = nc.alloc_sbuf_tensor("AT", [ROWS, W], F16).ap()
        covr = nc.alloc_sbuf_tensor("covr", [ROWS, 2 * 1792], F16).ap()
        wr = nc.alloc_sbuf_tensor("wr", [ROWS, 8 * WCAP], F16).ap()
        mr = nc.alloc_sbuf_tensor("mr", [ROWS, 4 * 3 * WCAP], F16).ap()
        scal_sb = nc.alloc_sbuf_tensor("scal_sb", [ROWS, S * 4], F32).ap()
        offs_sb = nc.alloc_sbuf_tensor("offs_sb", [1, S], I32).ap()

        nc.vector.memset(T, 1.0)
        nc.gpsimd.memset(CC, 0.0)

        CC3 = CC.rearrange("p (c x) -> p c x", x=W)
        mr3 = mr.rearrange("p (s x) -> p s x", x=WCAP)

        with (
            tc.tile_pool(name="psum", bufs=2, space="PSUM") as psum_pool,
            tc.tile_pool(name="ops", bufs=3) as oppool,
        ):
            pend = None  # (slot k, width, pool-offset) awaiting C adds
            for g in range(ngroups):
                k0 = g * 8
                p0 = k0 // 2
                gsize = min(8, S - k0)
                gp = gsize // 2
                gw = pair_start[p0 + gp] - pair_start[p0]
                lh_t = oppool.tile([8, 4 * ROWS], BF16, tag="lh")
                rh_t = oppool.tile([8, 1792], BF16, tag="rh")
                nc.sync.dma_start(lh_t[:, :gp * ROWS],
                                  lhsT_d[:, p0 * ROWS:(p0 + gp) * ROWS])
                nc.sync.dma_start(rh_t[:, :gw],
                                  rhs_d[:, pair_start[p0]:pair_start[p0 + gp]])
                if g == 0:
                    nc.sync.dma_start(offs_sb, offs_d)
                    nc.sync.dma_start(scal_sb, scal_d)
                pt = psum_pool.tile([ROWS, 4 * 512], F32)
                cbase = (g % 2) * 1792
                rpos = 0
                pair_pos = []
                for i in range(gp):
                    pw = pairw[p0 + i]
                    pair_pos.append(rpos)
                    nc.tensor.matmul(
                        pt[:, i * 512:i * 512 + pw],
                        lh_t[:, i * ROWS:(i + 1) * ROWS],
                        rh_t[:, rpos:rpos + pw],
                        start=True, stop=True)
                    rpos += pw

                # offsets for the group on Pool (w) and DVE (T chain, C adds)
                gregs = [nc.gpsimd.alloc_register(f"off_{k0}_{i}")
                         for i in range(gsize)]
                nc.gpsimd.reg_load(gregs, offs_sb[0:1, k0:k0 + gsize])
                goff = [nc.gpsimd.snap(gregs[j], donate=True, min_val=0,
                                       max_val=W - slotw[k0 + j])
                        for j in range(gsize)]
                vregs = [nc.vector.alloc_register(f"voff_{k0}_{i}")
                         for i in range(gsize)]
                nc.vector.reg_load(vregs, offs_sb[0:1, k0:k0 + gsize])
                voff = [nc.vector.snap(vregs[j], donate=True, min_val=0,
                                       max_val=W - slotw[k0 + j])
                        for j in range(gsize)]

                rpos = 0
                for j in range(gsize):
                    k = k0 + j
                    vk = slotw[k]
                    if j % 2 == 0:
                        i = j // 2
                        pw = pairw[p0 + i]
                        nc.scalar.activation(
                            covr[:, cbase + pair_pos[i]:
                                 cbase + pair_pos[i] + pw],
                            pt[:, i * 512:i * 512 + pw], AF.Sigmoid)
                    cov = covr[:, cbase + rpos:cbase + rpos + vk]
                    rpos += vk
                    wv = wr[:, (k % 8) * WCAP:(k % 8) * WCAP + vk]
                    tw = T[:, bass.ds(goff[j], vk)]
                    # Pool: w = T*cov  (Pool's only slot op)
                    nc.gpsimd.tensor_tensor(wv, tw, cov, OP.mult)
                    # DVE: T = (-a)*w + T  (chain-critical, emit first)
                    twv = T[:, bass.ds(voff[j], vk)]
                    nc.vector.scalar_tensor_tensor(
                        twv, wv, scal_sb[:, k * 4:k * 4 + 1], twv,
                        OP.mult, OP.add)
                    if pend is not None:
                        kp, vp, offp = pend
                        qv = CC3[:, :, bass.ds(offp, vp)]
                        mp = mr3[:, (kp % 4) * 3:(kp % 4) * 3 + 3, :vp]
                        nc.vector.tensor_tensor(qv, qv, mp, OP.add)
                    # Pool: m_G = (a colG) * w ; ACT: m_B = (a colB) * w
                    mbG = ((k % 4) * 3 + 1) * WCAP
                    nc.gpsimd.tensor_scalar(
                        mr[:, mbG:mbG + vk], wv,
                        scal_sb[:, k * 4 + 2:k * 4 + 3], 0.0,
                        OP.mult, OP.add)
                    mbB = ((k % 4) * 3 + 2) * WCAP
                    nc.scalar.activation(
                        mr[:, mbB:mbB + vk], wv, AF.Copy,
                        scale=scal_sb[:, k * 4 + 3:k * 4 + 4])
                    # DVE: m_R = (a colR) * w  (tensor_scalar, 4x-eligible)
                    mbR = (k % 4) * 3 * WCAP
                    nc.vector.tensor_scalar(
                        mr[:, mbR:mbR + vk], wv,
                        scal_sb[:, k * 4 + 1:k * 4 + 2], 0.0,
                        OP.mult, OP.add)
                    pend = (k, vk, voff[j])

            if pend is not None:
                kp, vp, offp = pend
                qv = CC3[:, :, bass.ds(offp, vp)]
                mp = mr3[:, (kp % 4) * 3:(kp % 4) * 3 + 3, :vp]
                nc.vector.tensor_tensor(qv, qv, mp, OP.add)
                pend = None

        # A = 1 - T, then plane DMAs
        nc.vector.tensor_scalar(AT, T, -1.0, 1.0, OP.mult, OP.add)
        nc.sync.dma_start(out_d[:, 0:3 * W], CC)
        nc.sync.dma_start(out_d[:, 3 * W:4 * W], AT)

    nc.compile()
    return nc


_CACHE = {}


def _get_nc(slotw):
    key = tuple(int(v) for v in slotw)
    if key not in _CACHE:
        _CACHE[key] = build_nc(slotw)
    return _CACHE[key]


def kernel(centers, radii, colors):
    centers = np.asarray(centers, np.float32)
    radii = np.asarray(radii, np.float32)
    colors = np.asarray(colors, np.float32)

    plan = make_plan(centers, radii)
    nc = _get_nc(plan[1])
    ins = make_inputs(centers, radii, colors, plan)
    res = bass_utils.run_bass_kernel_spmd(nc, ins, list(range(N_CORES)),
                                          trace=False)
    out = np.empty((H, W, 4), np.float32)
    for c in range(N_CORES):
        planes = res.results[c]["out"].astype(np.float32)  # [128, 4*W]
        for ch in range(4):
            out[c * ROWS:(c + 1) * ROWS, :, ch] = planes[:, ch * W:(ch + 1) * W]
    return out

